# revision 14
# baseline (speedup 1.0000x reference)
"""Caser model (vertical+horizontal conv recommender) on Trainium2 via Bass/Tile.

Data-parallel over 8 NeuronCores: each core handles B/8 = 1024 batch rows.
Embedding tables and weights are replicated per core; no collectives.

Per-core compute layout is "transposed" (batch on the matmul free dim):
  - gather Q[seq] rows (natural layout), PE-transpose into xT_l = [d=128, b]
  - vertical conv as K=16 block-diagonal matmuls producing [(f,par)=128, b]
    per d-pair (d, d+64), relu (ACT/DVE), then FC over (f,d) as K=128 matmuls
  - horizontal convs as K=128 matmuls with window-paired weight blocks
    (even-h in psum partitions 0:64, odd-h in 64:128), max over windows on
    DVE, relu+bias on ACT, then FC over (h,f)
  - z = relu(FC + bfc), PE-transpose back to natural [b, m], dot with
    gathered Qp[item] + P[user] rows on DVE, add b_item.
All matmuls run as float32r (fp32 bits, 1 cyc/col at N>=256).
"""

import numpy as np
from contextlib import ExitStack

try:
    import concourse.bass as bass
except ImportError:  # pragma: no cover
    import sys

    sys.path.insert(0, "/opt/trn_rl_repo")
    import concourse.bass as bass

import concourse.bacc as bacc
import concourse.mybir as mybir
import concourse.tile as tile
from concourse.bass_utils import run_bass_kernel_spmd
from concourse.masks import make_identity

F32 = mybir.dt.float32
F32R = mybir.dt.float32r
I32 = mybir.dt.int32

NUM_ITEMS = 100000
NUM_USERS = 100000
D = 128
L = 8
FV = 64
FH = 64
B_FULL = 8192
N_CORES = 8
B_CORE = B_FULL // N_CORES  # 1024
BT = 512  # batch tile (matmul free dim)

# ---------------------------------------------------------------------------
# Horizontal-conv pairing schedule.
# Classes pair an even h (psum partitions 0:64) with h-1 (64:128).
# Within a class, window j of h_even pairs with window j of h_odd; both
# windows read x positions l in [j, j+h). A job is one psum accumulation
# chain over l in [j, j+span).
# ---------------------------------------------------------------------------
H_CLASSES = [(2, 1), (4, 3), (6, 5), (8, 7)]


def _h_jobs():
    jobs = []  # (cls_idx, j, l0, nl, hasA)
    for ci, (he, ho) in enumerate(H_CLASSES):
        nA = L - he + 1
        nB = L - ho + 1
        for j in range(nB):  # nB >= nA always (ho = he-1)
            has_a = j < nA
            span = he if has_a else ho
            jobs.append((ci, j, j, span, has_a))
    return jobs


H_JOBS = _h_jobs()
H_STEPS = sum(j[3] for j in H_JOBS)  # total paired matmuls per b-tile (76)


def _pack_weights(Wv, bv, Wh, bh, Wfc, bfc):
    """Host-side weight packing into the SBUF layouts the kernel expects."""
    f32 = np.float32
    out = {}
    # vertical conv lhsT: K=16 rows (2l+par) -> [f + 64*par], replicated at
    # partition bases 0/32/64/96 (one per d-slot s; matmul row tile_position)
    wv_pack = np.zeros((128, 128), f32)
    for s in range(4):
        for l in range(L):
            for par in range(2):
                wv_pack[32 * s + 2 * l + par, 64 * par : 64 * par + 64] = Wv[:, l]
    out["WvPack"] = wv_pack
    out["bv2"] = np.concatenate([bv, bv]).reshape(128, 1).astype(f32)

    # horizontal conv paired blocks: per step one contiguous [128 d, 128]
    # lhsT block: cols 0:64 = even-h window (A), 64:128 = odd-h (B).
    whab = np.zeros((128, H_STEPS, 128), f32)
    step = 0
    for ci, j, l0, nl, has_a in H_JOBS:
        he, ho = H_CLASSES[ci]
        for s in range(nl):
            if has_a and s < he:
                whab[:, step, 0:64] = Wh[he - 1, :, s, :].T
            if s < ho:
                whab[:, step, 64:128] = Wh[ho - 1, :, s, :].T
            step += 1
    assert step == H_STEPS
    out["WhPair"] = whab.reshape(128, H_STEPS * 128)
    bh_pair = np.zeros((128, len(H_CLASSES)), f32)
    for ci, (he, ho) in enumerate(H_CLASSES):
        bh_pair[0:64, ci] = bh[he - 1]
        bh_pair[64:128, ci] = bh[ho - 1]
    out["bhPair"] = bh_pair

    # FC over out_v: lhsT per d-pair t (covers d=t and d=64+t):
    # [128=(f + 64*par), 128=m]
    wfc_v = Wfc[:, : FV * D].reshape(D, FV, D)  # [m, f, d]
    wfcv = np.zeros((128, FV * 2, D // 2), f32)  # [k, ...] -> build per t
    wfcv_t = np.zeros((128, D // 2, 128), f32)  # [k, t, m]
    for t in range(D // 2):
        wfcv_t[0:64, t, :] = wfc_v[:, :, t].T
        wfcv_t[64:128, t, :] = wfc_v[:, :, 64 + t].T
    out["WfcvT"] = wfcv_t.reshape(128, (D // 2) * 128)

    # FC over out_h: lhsT per class ci: [128=(f_even | f_odd), 128=m]
    wfch_t = np.zeros((128, len(H_CLASSES), 128), f32)
    for ci, (he, ho) in enumerate(H_CLASSES):
        wfch_t[0:64, ci, :] = Wfc[:, FV * D + (he - 1) * 64 : FV * D + he * 64].T
        wfch_t[64:128, ci, :] = Wfc[:, FV * D + (ho - 1) * 64 : FV * D + ho * 64].T
    out["WfchT"] = wfch_t.reshape(128, len(H_CLASSES) * 128)

    out["bfc1"] = bfc.reshape(128, 1).astype(f32)
    return out


def build_program(b_core=B_CORE, bt=BT, v_items=NUM_ITEMS, v_users=NUM_USERS):
    """Build the per-core Bass program. Returns the compiled Bass object."""
    nc = bacc.Bacc("TRN2", target_bir_lowering=False, debug=False)

    # ---- DRAM parameters -------------------------------------------------
    Qd = nc.dram_tensor("Q", [v_items, D], F32, kind="ExternalInput").ap()
    Pd = nc.dram_tensor("P", [v_users, D], F32, kind="ExternalInput").ap()
    Qpd = nc.dram_tensor("Qp", [v_items, 2 * D], F32, kind="ExternalInput").ap()
    bid = nc.dram_tensor("b_item", [v_items, 1], F32, kind="ExternalInput").ap()
    seqT = nc.dram_tensor("seqT32", [L, b_core], I32, kind="ExternalInput").ap()
    usr = nc.dram_tensor("user32", [b_core, 1], I32, kind="ExternalInput").ap()
    itm = nc.dram_tensor("item32", [b_core, 1], I32, kind="ExternalInput").ap()
    WvP = nc.dram_tensor("WvPack", [128, 128], F32R, kind="ExternalInput").ap()
    bv2 = nc.dram_tensor("bv2", [128, 1], F32, kind="ExternalInput").ap()
    WhP = nc.dram_tensor(
        "WhPair", [128, H_STEPS * 128], F32R, kind="ExternalInput"
    ).ap()
    bhP = nc.dram_tensor("bhPair", [128, len(H_CLASSES)], F32, kind="ExternalInput").ap()
    Wfv = nc.dram_tensor("WfcvT", [128, (D // 2) * 128], F32R, kind="ExternalInput").ap()
    Wfh = nc.dram_tensor(
        "WfchT", [128, len(H_CLASSES) * 128], F32R, kind="ExternalInput"
    ).ap()
    bfc1 = nc.dram_tensor("bfc1", [128, 1], F32, kind="ExternalInput").ap()
    score = nc.dram_tensor("score", [b_core, 1], F32, kind="ExternalOutput").ap()

    n_bt = b_core // bt
    n_ch = bt // 128  # 128-row chunks per b-tile

    with tile.TileContext(nc) as tc, ExitStack() as ctx:
        singles = ctx.enter_context(tc.tile_pool(name="singles", bufs=1))
        gpool = ctx.enter_context(tc.tile_pool(name="gath", bufs=6))
        idxp = ctx.enter_context(tc.tile_pool(name="idx", bufs=6))
        xtp = ctx.enter_context(tc.tile_pool(name="xt", bufs=2))
        b2p = ctx.enter_context(tc.tile_pool(name="b2", bufs=2))
        rvp = ctx.enter_context(tc.tile_pool(name="rv", bufs=3))
        ohp = ctx.enter_context(tc.tile_pool(name="oh", bufs=2))
        zsp = ctx.enter_context(tc.tile_pool(name="zs", bufs=2))
        scp = ctx.enter_context(tc.tile_pool(name="sc", bufs=2))
        ps_x = ctx.enter_context(tc.tile_pool(name="psx", bufs=2, space="PSUM"))
        ps_v = ctx.enter_context(tc.tile_pool(name="psv", bufs=2, space="PSUM"))
        ps_h = ctx.enter_context(tc.tile_pool(name="psh", bufs=2, space="PSUM"))
        ps_z = ctx.enter_context(tc.tile_pool(name="psz", bufs=1, space="PSUM"))
        ps_t = ctx.enter_context(tc.tile_pool(name="pst", bufs=1, space="PSUM"))

        # ---- persistent weights in SBUF ---------------------------------
        ident = singles.tile([128, 128], F32)
        make_identity(nc, ident)
        wv_sb = singles.tile([128, 128], F32R)
        nc.sync.dma_start(out=wv_sb, in_=WvP)
        bv_sb = singles.tile([128, 1], F32)
        nc.sync.dma_start(out=bv_sb, in_=bv2)
        wh_sb = singles.tile([128, H_STEPS * 128], F32R)
        nc.sync.dma_start(out=wh_sb, in_=WhP)
        bh_sb = singles.tile([128, len(H_CLASSES)], F32)
        nc.sync.dma_start(out=bh_sb, in_=bhP)
        wfv_sb = singles.tile([128, (D // 2) * 128], F32R)
        nc.sync.dma_start(out=wfv_sb, in_=Wfv)
        wfh_sb = singles.tile([128, len(H_CLASSES) * 128], F32R)
        nc.sync.dma_start(out=wfh_sb, in_=Wfh)
        bfc_sb = singles.tile([128, 1], F32)
        nc.sync.dma_start(out=bfc_sb, in_=bfc1)

        wh_view = wh_sb[:].rearrange("p (q c) -> p q c", c=128)

        for ib in range(n_bt):
            b0 = ib * bt

            # ---- gather + transpose x into xT_l = [d, b] ----------------
            xts = []
            for l in range(L):
                xt_ps = ps_x.tile([128, bt], F32, tag="xtp")
                for c in range(n_ch):
                    idx_t = idxp.tile([128, 1], I32, tag="idx")
                    nc.sync.dma_start(
                        out=idx_t, in_=seqT[l, b0 + c * 128 : b0 + (c + 1) * 128]
                    )
                    g_t = gpool.tile([128, D], F32, tag="g")
                    nc.gpsimd.indirect_dma_start(
                        out=g_t[:],
                        out_offset=None,
                        in_=Qd,
                        in_offset=bass.IndirectOffsetOnAxis(ap=idx_t[:, 0:1], axis=0),
                    )
                    nc.tensor.transpose(
                        xt_ps[:, c * 128 : (c + 1) * 128], g_t[:], ident[:]
                    )
                xt_l = xtp.tile([128, bt], F32R, tag=f"xt{l}")
                nc.scalar.copy(out=xt_l[:], in_=xt_ps[:])
                xts.append(xt_l)

            # ---- vconv rhs layout: partition 32s + (2l+par) holds
            # (g=0..15, b) with d = par*64 + s*16 + g ------------------------
            b3_t = b2p.tile([128, 16, bt], F32R, tag="b3")
            for s4 in range(4):
                for l in range(L):
                    for par in range(2):
                        p0 = 32 * s4 + 2 * l + par
                        d0 = par * 64 + s4 * 16
                        nc.sync.dma_start(
                            out=b3_t[p0 : p0 + 1, :, :],
                            in_=xts[l][d0 : d0 + 16, :],
                        )

            # ---- vertical conv + relu + FC_v accumulation ----------------
            z_ps = ps_z.tile([128, bt], F32, tag="zp")
            n_fc = D // 2 + len(H_CLASSES)  # total matmuls into z_ps
            fc_i = 0
            for t in range(D // 2):
                s4, g = t // 16, t % 16
                v_ps = ps_v.tile([128, bt], F32, tag="vp")
                rhs = b3_t[32 * s4 : 32 * s4 + 16, g, :]
                lhs = wv_sb[32 * s4 : 32 * s4 + 16, :]
                nc.tensor.matmul(
                    v_ps[:], lhs, rhs, start=True, stop=True,
                    tile_position=(32 * s4, 0),
                )
                rv_t = rvp.tile([128, bt], F32R, tag="rv")
                if t % 2 == 0:
                    nc.scalar.activation(
                        rv_t[:], v_ps[:], mybir.ActivationFunctionType.Relu,
                        bias=bv_sb[:, 0:1],
                    )
                else:
                    nc.vector.tensor_scalar(
                        out=rv_t[:], in0=v_ps[:], scalar1=bv_sb[:, 0:1],
                        scalar2=0.0, op0=mybir.AluOpType.add,
                        op1=mybir.AluOpType.max,
                    )
                nc.tensor.matmul(
                    z_ps[:],
                    wfv_sb[:, t * 128 : (t + 1) * 128],
                    rv_t[:],
                    start=(fc_i == 0),
                    stop=(fc_i == n_fc - 1),
                )
                fc_i += 1

            # ---- horizontal convs ---------------------------------------
            oh_tiles = []
            step = 0
            job_ix = 0
            cls_jobs = {}
            for ci, j, l0, nl, has_a in H_JOBS:
                cls_jobs.setdefault(ci, []).append((j, l0, nl, has_a, step))
                step += nl
            for ci in range(len(H_CLASSES)):
                oh_t = ohp.tile([128, bt], F32R, tag=f"oh{ci}")
                first = True
                for (j, l0, nl, has_a, st0) in cls_jobs[ci]:
                    h_ps = ps_h.tile([128, bt], F32, tag="hp")
                    for s in range(nl):
                        lhs = wh_view[:, st0 + s, :]
                        nc.tensor.matmul(
                            h_ps[:],
                            lhs,
                            xts[l0 + s][:],
                            start=(s == 0),
                            stop=(s == nl - 1),
                        )
                    if first:
                        nc.vector.tensor_copy(out=oh_t[:], in_=h_ps[:])
                        first = False
                    elif has_a:
                        nc.vector.tensor_tensor(
                            out=oh_t[:], in0=oh_t[:], in1=h_ps[:],
                            op=mybir.AluOpType.max,
                        )
                    else:
                        nc.vector.tensor_tensor(
                            out=oh_t[64:128, :], in0=oh_t[64:128, :],
                            in1=h_ps[64:128, :], op=mybir.AluOpType.max,
                        )
                nc.scalar.activation(
                    oh_t[:], oh_t[:], mybir.ActivationFunctionType.Relu,
                    bias=bh_sb[:, ci : ci + 1],
                )
                oh_tiles.append(oh_t)

            # ---- FC_h into z ---------------------------------------------
            for ci in range(len(H_CLASSES)):
                nc.tensor.matmul(
                    z_ps[:],
                    wfh_sb[:, ci * 128 : (ci + 1) * 128],
                    oh_tiles[ci][:],
                    start=(fc_i == 0),
                    stop=(fc_i == n_fc - 1),
                )
                fc_i += 1

            # ---- z = relu(. + bfc), transpose to natural -----------------
            z_sb = zsp.tile([128, bt], F32, tag="zsb")
            nc.scalar.activation(
                z_sb[:], z_ps[:], mybir.ActivationFunctionType.Relu,
                bias=bfc_sb[:, 0:1],
            )
            zt_ps = ps_t.tile([128, bt], F32, tag="ztp")
            for c in range(n_ch):
                nc.tensor.transpose(
                    zt_ps[:, c * 128 : (c + 1) * 128],
                    z_sb[:, c * 128 : (c + 1) * 128],
                    ident[:],
                )

            # ---- scoring -------------------------------------------------
            for c in range(n_ch):
                r0 = b0 + c * 128
                ui = idxp.tile([128, 1], I32, tag="uix")
                nc.sync.dma_start(out=ui, in_=usr[r0 : r0 + 128, :])
                ii = idxp.tile([128, 1], I32, tag="iix")
                nc.sync.dma_start(out=ii, in_=itm[r0 : r0 + 128, :])
                p_g = scp.tile([128, D], F32, tag="pg")
                nc.gpsimd.indirect_dma_start(
                    out=p_g[:], out_offset=None, in_=Pd,
                    in_offset=bass.IndirectOffsetOnAxis(ap=ui[:, 0:1], axis=0),
                )
                qp_g = scp.tile([128, 2 * D], F32, tag="qpg")
                nc.gpsimd.indirect_dma_start(
                    out=qp_g[:], out_offset=None, in_=Qpd,
                    in_offset=bass.IndirectOffsetOnAxis(ap=ii[:, 0:1], axis=0),
                )
                bi_g = scp.tile([128, 1], F32, tag="big")
                nc.gpsimd.indirect_dma_start(
                    out=bi_g[:], out_offset=None, in_=bid,
                    in_offset=bass.IndirectOffsetOnAxis(ap=ii[:, 0:1], axis=0),
                )
                sc_t = scp.tile([128, 2 * D], F32, tag="sct")
                nc.vector.tensor_tensor(
                    out=sc_t[:, 0:D], in0=zt_ps[:, c * 128 : (c + 1) * 128],
                    in1=qp_g[:, 0:D], op=mybir.AluOpType.mult,
                )
                nc.vector.tensor_tensor(
                    out=sc_t[:, D : 2 * D], in0=p_g[:], in1=qp_g[:, D : 2 * D],
                    op=mybir.AluOpType.mult,
                )
                s_t = scp.tile([128, 1], F32, tag="st")
                nc.vector.reduce_sum(out=s_t[:], in_=sc_t[:], axis=mybir.AxisListType.X)
                so_t = scp.tile([128, 1], F32, tag="sot")
                nc.vector.tensor_tensor(
                    out=so_t[:], in0=s_t[:], in1=bi_g[:], op=mybir.AluOpType.add
                )
                nc.sync.dma_start(out=score[r0 : r0 + 128, :], in_=so_t[:])

    nc.compile()
    return nc


_NC_CACHE = {}


def _get_nc(key):
    if key not in _NC_CACHE:
        _NC_CACHE[key] = build_program(*key)
    return _NC_CACHE[key]


def make_in_maps(inputs, b_core=B_CORE, n_cores=N_CORES):
    """Shard inputs into per-core input maps (host side)."""
    w = _pack_weights(
        np.asarray(inputs["Wv"], np.float32),
        np.asarray(inputs["bv"], np.float32),
        np.asarray(inputs["Wh"], np.float32),
        np.asarray(inputs["bh"], np.float32),
        np.asarray(inputs["Wfc"], np.float32),
        np.asarray(inputs["bfc"], np.float32),
    )
    Q = np.ascontiguousarray(np.asarray(inputs["Q"], np.float32))
    P = np.ascontiguousarray(np.asarray(inputs["P"], np.float32))
    Qp = np.ascontiguousarray(np.asarray(inputs["Qp"], np.float32))
    bi = np.ascontiguousarray(
        np.asarray(inputs["b_item"], np.float32).reshape(-1, 1)
    )
    seq = np.asarray(inputs["seq_L"]).astype(np.int32)  # [B, L]
    user = np.asarray(inputs["user_id"]).astype(np.int32)
    item = np.asarray(inputs["item_id"]).astype(np.int32)

    in_maps = []
    for ci in range(n_cores):
        s = slice(ci * b_core, (ci + 1) * b_core)
        m = dict(w)
        m["Q"] = Q
        m["P"] = P
        m["Qp"] = Qp
        m["b_item"] = bi
        m["seqT32"] = np.ascontiguousarray(seq[s].T)  # [L, b_core]
        m["user32"] = np.ascontiguousarray(user[s].reshape(-1, 1))
        m["item32"] = np.ascontiguousarray(item[s].reshape(-1, 1))
        in_maps.append(m)
    return in_maps


def kernel(**inputs):
    nc = _get_nc((B_CORE, BT, NUM_ITEMS, NUM_USERS))
    in_maps = make_in_maps(inputs)
    res = run_bass_kernel_spmd(nc, in_maps, core_ids=list(range(N_CORES)))
    out = np.concatenate(
        [np.asarray(res.results[i]["score"]).reshape(-1) for i in range(N_CORES)]
    )
    return out.astype(np.float32)


# revision 23
# speedup vs baseline: 1.1984x; 1.1984x over previous
"""Caser model (vertical+horizontal conv recommender) on Trainium2 via Bass/Tile.

Data-parallel over 8 NeuronCores: each core handles B/8 = 1024 batch rows.
Embedding tables and weights are replicated per core; no collectives.

Per-core compute layout is "transposed" (batch on the matmul free dim):
  - gather Q[seq] rows (bf16, natural layout), PE-transpose into xT_l = [d, b]
  - vertical conv as K=16 block-diagonal matmuls producing [(f,par)=128, b]
    per d-pair (d, d+64), relu (ACT/DVE), then FC over (f,d) as K=128 matmuls
  - horizontal convs as K=128 matmuls with window-paired weight blocks
    (even-h in psum partitions 0:64, odd-h in 64:128), max over windows on
    DVE, relu+bias on ACT, then FC over (h,f)
  - z = relu(FC + bfc) in f32, PE-transpose back to natural [b, m], dot with
    gathered Qp[item] + P[user] rows on DVE, add b_item.
Conv/FC matmuls run in bf16 (f32 PSUM accumulation); scoring stays f32.
"""

import numpy as np
from contextlib import ExitStack

try:
    import concourse.bass as bass
except ImportError:  # pragma: no cover
    import sys

    sys.path.insert(0, "/opt/trn_rl_repo")
    import concourse.bass as bass

import ml_dtypes
import concourse.bacc as bacc
import concourse.mybir as mybir
import concourse.tile as tile
from concourse.bass_utils import run_bass_kernel_spmd

F32 = mybir.dt.float32
I32 = mybir.dt.int32

USE_BF16 = True
if USE_BF16:
    CDT = mybir.dt.bfloat16
    NP_CDT = ml_dtypes.bfloat16
else:
    CDT = mybir.dt.float32r
    NP_CDT = np.float32

NUM_ITEMS = 100000
NUM_USERS = 100000
D = 128
L = 8
FV = 64
FH = 64
B_FULL = 8192
N_CORES = 8
B_CORE = B_FULL // N_CORES  # 1024
BT = 512  # batch tile (matmul free dim)

# ---------------------------------------------------------------------------
# Horizontal-conv pairing schedule.
# Classes pair an even h (psum partitions 0:64) with h-1 (64:128).
# Window j of h_even pairs with window j of h_odd; a job is one psum
# accumulation chain over l in [j, j+span).
# ---------------------------------------------------------------------------
H_CLASSES = [(2, 1), (4, 3), (6, 5), (8, 7)]


def _h_jobs():
    jobs = []  # (cls_idx, j, l0, nl, hasA)
    for ci, (he, ho) in enumerate(H_CLASSES):
        nA = L - he + 1
        nB = L - ho + 1
        for j in range(nB):  # nB >= nA always (ho = he-1)
            has_a = j < nA
            span = he if has_a else ho
            jobs.append((ci, j, j, span, has_a))
    return jobs


H_JOBS = _h_jobs()
H_STEPS = sum(j[3] for j in H_JOBS)  # total paired matmuls per b-tile (76)
CLS_STEPS = [sum(j[3] for j in H_JOBS if j[0] == ci) for ci in range(len(H_CLASSES))]


def _pack_weights(Wv, bv, Wh, bh, Wfc, bfc):
    """Host-side weight packing into the SBUF layouts the kernel expects."""
    f32 = np.float32
    out = {}
    # vertical conv lhsT: K=16 rows (2l+par) -> [f + 64*par], replicated at
    # partition bases 0/32/64/96 (one per d-slot s; matmul row tile_position)
    wv_pack = np.zeros((128, 128), f32)
    for s in range(4):
        for l in range(L):
            for par in range(2):
                wv_pack[32 * s + 2 * l + par, 64 * par : 64 * par + 64] = Wv[:, l]
    out["WvPack"] = wv_pack.astype(NP_CDT)
    out["bv2"] = np.concatenate([bv, bv]).reshape(128, 1).astype(f32)

    # horizontal conv paired blocks: per step one contiguous [128 d, 128]
    # lhsT block: cols 0:64 = even-h window (A), 64:128 = odd-h (B).
    whab = np.zeros((128, H_STEPS, 128), f32)
    step = 0
    for ci, j, l0, nl, has_a in H_JOBS:
        he, ho = H_CLASSES[ci]
        for s in range(nl):
            if has_a and s < he:
                whab[:, step, 0:64] = Wh[he - 1, :, s, :].T
            if s < ho:
                whab[:, step, 64:128] = Wh[ho - 1, :, s, :].T
            step += 1
    assert step == H_STEPS
    out["WhPair"] = whab.reshape(128, H_STEPS * 128).astype(NP_CDT)
    bh_pair = np.zeros((128, len(H_CLASSES)), f32)
    for ci, (he, ho) in enumerate(H_CLASSES):
        bh_pair[0:64, ci] = bh[he - 1]
        bh_pair[64:128, ci] = bh[ho - 1]
    out["bhPair"] = bh_pair

    # FC over out_v: lhsT per d-pair t (covers d=t and d=64+t):
    # [128=(f + 64*par), 128=m]
    wfc_v = Wfc[:, : FV * D].reshape(D, FV, D)  # [m, f, d]
    wfcv_t = np.zeros((128, D // 2, 128), f32)  # [k, t, m]
    for t in range(D // 2):
        wfcv_t[0:64, t, :] = wfc_v[:, :, t].T
        wfcv_t[64:128, t, :] = wfc_v[:, :, 64 + t].T
    out["WfcvT"] = wfcv_t.reshape(128, (D // 2) * 128).astype(NP_CDT)

    # FC over out_h: lhsT per class ci: [128=(f_even | f_odd), 128=m]
    wfch_t = np.zeros((128, len(H_CLASSES), 128), f32)
    for ci, (he, ho) in enumerate(H_CLASSES):
        wfch_t[0:64, ci, :] = Wfc[:, FV * D + (he - 1) * 64 : FV * D + he * 64].T
        wfch_t[64:128, ci, :] = Wfc[:, FV * D + (ho - 1) * 64 : FV * D + ho * 64].T
    out["WfchT"] = wfch_t.reshape(128, len(H_CLASSES) * 128).astype(NP_CDT)

    out["bfc1"] = bfc.reshape(128, 1).astype(f32)
    out["ident"] = np.eye(128, dtype=f32).astype(NP_CDT)
    return out


def build_program(b_core=B_CORE, bt=BT, v_items=NUM_ITEMS, v_users=NUM_USERS):
    """Build the per-core Bass program. Returns the compiled Bass object."""
    nc = bacc.Bacc("TRN2", target_bir_lowering=False, debug=False)

    n_cls = len(H_CLASSES)

    # ---- DRAM parameters -------------------------------------------------
    Qd = nc.dram_tensor("Q", [v_items, D], CDT, kind="ExternalInput").ap()
    Pd = nc.dram_tensor("P", [v_users, D], F32, kind="ExternalInput").ap()
    Qpd = nc.dram_tensor("Qp", [v_items, 2 * D], F32, kind="ExternalInput").ap()
    bid = nc.dram_tensor("b_item", [v_items, 1], F32, kind="ExternalInput").ap()
    seqT = nc.dram_tensor("seqT32", [L, b_core], I32, kind="ExternalInput").ap()
    usr = nc.dram_tensor("user32", [b_core], I32, kind="ExternalInput").ap()
    itm = nc.dram_tensor("item32", [b_core], I32, kind="ExternalInput").ap()
    WvP = nc.dram_tensor("WvPack", [128, 128], CDT, kind="ExternalInput").ap()
    bv2 = nc.dram_tensor("bv2", [128, 1], F32, kind="ExternalInput").ap()
    WhP = nc.dram_tensor(
        "WhPair", [128, H_STEPS * 128], CDT, kind="ExternalInput"
    ).ap()
    bhP = nc.dram_tensor("bhPair", [128, n_cls], F32, kind="ExternalInput").ap()
    Wfv = nc.dram_tensor("WfcvT", [128, (D // 2) * 128], CDT, kind="ExternalInput").ap()
    Wfh = nc.dram_tensor("WfchT", [128, n_cls * 128], CDT, kind="ExternalInput").ap()
    bfc1 = nc.dram_tensor("bfc1", [128, 1], F32, kind="ExternalInput").ap()
    idnt = nc.dram_tensor("ident", [128, 128], CDT, kind="ExternalInput").ap()
    score = nc.dram_tensor("score", [b_core, 1], F32, kind="ExternalOutput").ap()

    n_bt = b_core // bt
    n_ch = bt // 128  # 128-row chunks per b-tile

    with tile.TileContext(nc) as tc, ExitStack() as ctx:
        singles = ctx.enter_context(tc.tile_pool(name="singles", bufs=1))
        gpool = ctx.enter_context(tc.tile_pool(name="gath", bufs=3))
        idxp = ctx.enter_context(tc.tile_pool(name="idx", bufs=3))
        xtp = ctx.enter_context(tc.tile_pool(name="xt", bufs=2 if USE_BF16 else 1))
        b2p = ctx.enter_context(tc.tile_pool(name="b2", bufs=1))
        rvp = ctx.enter_context(tc.tile_pool(name="rv", bufs=4))
        ohp = ctx.enter_context(tc.tile_pool(name="oh", bufs=2))
        zsp = ctx.enter_context(tc.tile_pool(name="zs", bufs=2))
        scp = ctx.enter_context(tc.tile_pool(name="sc", bufs=2))
        ps_x = ctx.enter_context(tc.tile_pool(name="psx", bufs=2, space="PSUM"))
        ps_v = ctx.enter_context(tc.tile_pool(name="psv", bufs=2, space="PSUM"))
        ps_h = ctx.enter_context(tc.tile_pool(name="psh", bufs=2, space="PSUM"))
        ps_z = ctx.enter_context(tc.tile_pool(name="psz", bufs=1, space="PSUM"))
        ps_t = ctx.enter_context(tc.tile_pool(name="pst", bufs=1, space="PSUM"))

        # ---- persistent weights in SBUF ---------------------------------
        ident = singles.tile([128, 128], CDT)
        nc.sync.dma_start(out=ident, in_=idnt)
        wv_sb = singles.tile([128, 128], CDT)
        nc.sync.dma_start(out=wv_sb, in_=WvP)
        bv_sb = singles.tile([128, 1], F32)
        nc.sync.dma_start(out=bv_sb, in_=bv2)
        # horizontal conv weights, one tile per class (early start)
        wh_cls = []
        off = 0
        for ci in range(n_cls):
            w = singles.tile([128, CLS_STEPS[ci] * 128], CDT, tag=f"whc{ci}")
            nc.sync.dma_start(
                out=w, in_=WhP[:, off * 128 : (off + CLS_STEPS[ci]) * 128]
            )
            wh_cls.append(w[:].rearrange("p (q c) -> p q c", c=128))
            off += CLS_STEPS[ci]
        bh_sb = singles.tile([128, n_cls], F32)
        nc.sync.dma_start(out=bh_sb, in_=bhP)
        # FC_v weights in 4 chunks of 16 d-pairs
        wfv_sb = []
        for q in range(4):
            w = singles.tile([128, 16 * 128], CDT, tag=f"wfv{q}")
            nc.sync.dma_start(out=w, in_=Wfv[:, q * 16 * 128 : (q + 1) * 16 * 128])
            wfv_sb.append(w)
        wfh_sb = singles.tile([128, n_cls * 128], CDT)
        nc.sync.dma_start(out=wfh_sb, in_=Wfh)
        bfc_sb = singles.tile([128, 1], F32)
        nc.sync.dma_start(out=bfc_sb, in_=bfc1)

        for ib in range(n_bt):
            b0 = ib * bt

            # ---- gather + transpose x into xT_l = [d, b] ----------------
            xts = []
            for l in range(L):
                idx_t = idxp.tile([128, n_ch], I32, tag="idx")
                g_t = gpool.tile([128, n_ch, D], CDT, tag="g")
                for c in range(n_ch):
                    nc.sync.dma_start(
                        out=idx_t[:, c : c + 1],
                        in_=seqT[l, b0 + c * 128 : b0 + (c + 1) * 128],
                    )
                    nc.gpsimd.indirect_dma_start(
                        out=g_t[:, c, :],
                        out_offset=None,
                        in_=Qd,
                        in_offset=bass.IndirectOffsetOnAxis(
                            ap=idx_t[:, c : c + 1], axis=0
                        ),
                    )
                xt_ps = ps_x.tile([128, bt], CDT, tag="xtp")
                for c in range(n_ch):
                    nc.tensor.transpose(
                        xt_ps[:, c * 128 : (c + 1) * 128], g_t[:, c, :], ident[:]
                    )
                xt_l = xtp.tile([128, bt], CDT, tag=f"xt{l}")
                nc.scalar.copy(out=xt_l[:], in_=xt_ps[:])
                xts.append(xt_l)

            # ---- vconv rhs layout: partition 32s + (2l+par) holds
            # (g=0..15, b) with d = par*64 + s*16 + g ----------------------
            b3_t = b2p.tile([128, 16, bt], CDT, tag=f"b3_{ib % 2 if USE_BF16 else 0}")
            for s4 in range(4):
                for l in range(L):
                    for par in range(2):
                        p0 = 32 * s4 + 2 * l + par
                        d0 = par * 64 + s4 * 16
                        eng = nc.sync if (l + par) % 2 == 0 else nc.scalar
                        eng.dma_start(
                            out=b3_t[p0 : p0 + 1, :, :],
                            in_=xts[l][d0 : d0 + 16, :],
                        )

            # ---- vertical conv + relu + FC_v accumulation ----------------
            z_ps = ps_z.tile([128, bt], F32, tag="zp")
            n_fc = D // 2 + n_cls  # total matmuls into z_ps
            fc_i = 0
            for t in range(D // 2):
                s4, g = t // 16, t % 16
                v_ps = ps_v.tile([128, bt], F32, tag="vp")
                rhs = b3_t[32 * s4 : 32 * s4 + 16, g, :]
                lhs = wv_sb[32 * s4 : 32 * s4 + 16, :]
                nc.tensor.matmul(
                    v_ps[:], lhs, rhs, start=True, stop=True,
                    tile_position=(32 * s4, 0),
                )
                rv_t = rvp.tile([128, bt], CDT, tag="rv")
                if t % 2 == 0:
                    nc.scalar.activation(
                        rv_t[:], v_ps[:], mybir.ActivationFunctionType.Relu,
                        bias=bv_sb[:, 0:1],
                    )
                else:
                    nc.vector.tensor_scalar(
                        out=rv_t[:], in0=v_ps[:], scalar1=bv_sb[:, 0:1],
                        scalar2=0.0, op0=mybir.AluOpType.add,
                        op1=mybir.AluOpType.max,
                    )
                nc.tensor.matmul(
                    z_ps[:],
                    wfv_sb[s4][:, g * 128 : (g + 1) * 128],
                    rv_t[:],
                    start=(fc_i == 0),
                    stop=(fc_i == n_fc - 1),
                )
                fc_i += 1

            # ---- horizontal convs ---------------------------------------
            oh_tiles = []
            cls_jobs = {}
            step_in_cls = {}
            for ci, j, l0, nl, has_a in H_JOBS:
                st = step_in_cls.get(ci, 0)
                cls_jobs.setdefault(ci, []).append((j, l0, nl, has_a, st))
                step_in_cls[ci] = st + nl
            for ci in range(n_cls):
                oh_t = ohp.tile([128, bt], CDT, tag=f"oh{ci}")
                first = True
                for (j, l0, nl, has_a, st0) in cls_jobs[ci]:
                    h_ps = ps_h.tile([128, bt], F32, tag="hp")
                    for s in range(nl):
                        nc.tensor.matmul(
                            h_ps[:],
                            wh_cls[ci][:, st0 + s, :],
                            xts[l0 + s][:],
                            start=(s == 0),
                            stop=(s == nl - 1),
                        )
                    if first:
                        nc.vector.tensor_copy(out=oh_t[:], in_=h_ps[:])
                        first = False
                    elif has_a:
                        nc.vector.tensor_tensor(
                            out=oh_t[:], in0=oh_t[:], in1=h_ps[:],
                            op=mybir.AluOpType.max,
                        )
                    else:
                        nc.vector.tensor_tensor(
                            out=oh_t[64:128, :], in0=oh_t[64:128, :],
                            in1=h_ps[64:128, :], op=mybir.AluOpType.max,
                        )
                nc.scalar.activation(
                    oh_t[:], oh_t[:], mybir.ActivationFunctionType.Relu,
                    bias=bh_sb[:, ci : ci + 1],
                )
                oh_tiles.append(oh_t)

            # ---- FC_h into z ---------------------------------------------
            for ci in range(n_cls):
                nc.tensor.matmul(
                    z_ps[:],
                    wfh_sb[:, ci * 128 : (ci + 1) * 128],
                    oh_tiles[ci][:],
                    start=(fc_i == 0),
                    stop=(fc_i == n_fc - 1),
                )
                fc_i += 1

            # ---- z = relu(. + bfc), transpose to natural -----------------
            z_sb = zsp.tile([128, bt], CDT, tag="zsb")
            nc.scalar.activation(
                z_sb[:], z_ps[:], mybir.ActivationFunctionType.Relu,
                bias=bfc_sb[:, 0:1],
            )
            zt_ps = ps_t.tile([128, bt], CDT, tag="ztp")
            for c in range(n_ch):
                nc.tensor.transpose(
                    zt_ps[:, c * 128 : (c + 1) * 128],
                    z_sb[:, c * 128 : (c + 1) * 128],
                    ident[:],
                )

            # ---- scoring -------------------------------------------------
            ui = idxp.tile([128, n_ch], I32, tag="uix")
            ii = idxp.tile([128, n_ch], I32, tag="iix")
            p_g = scp.tile([128, n_ch, D], F32, tag="pg")
            qp_g = scp.tile([128, n_ch, 2 * D], F32, tag="qpg")
            bi_g = scp.tile([128, n_ch], F32, tag="big")
            for c in range(n_ch):
                r0 = b0 + c * 128
                nc.sync.dma_start(out=ui[:, c : c + 1], in_=usr[r0 : r0 + 128])
                nc.sync.dma_start(out=ii[:, c : c + 1], in_=itm[r0 : r0 + 128])
                nc.gpsimd.indirect_dma_start(
                    out=p_g[:, c, :], out_offset=None, in_=Pd,
                    in_offset=bass.IndirectOffsetOnAxis(ap=ui[:, c : c + 1], axis=0),
                )
                nc.gpsimd.indirect_dma_start(
                    out=qp_g[:, c, :], out_offset=None, in_=Qpd,
                    in_offset=bass.IndirectOffsetOnAxis(ap=ii[:, c : c + 1], axis=0),
                )
                nc.gpsimd.indirect_dma_start(
                    out=bi_g[:, c : c + 1], out_offset=None, in_=bid,
                    in_offset=bass.IndirectOffsetOnAxis(ap=ii[:, c : c + 1], axis=0),
                )
            for c in range(n_ch):
                r0 = b0 + c * 128
                sc_t = scp.tile([128, 2 * D], F32, tag="sct")
                nc.vector.tensor_tensor(
                    out=sc_t[:, 0:D], in0=zt_ps[:, c * 128 : (c + 1) * 128],
                    in1=qp_g[:, c, 0:D], op=mybir.AluOpType.mult,
                )
                nc.vector.tensor_tensor(
                    out=sc_t[:, D : 2 * D], in0=p_g[:, c, :],
                    in1=qp_g[:, c, D : 2 * D], op=mybir.AluOpType.mult,
                )
                s_t = scp.tile([128, 1], F32, tag="st")
                nc.vector.reduce_sum(out=s_t[:], in_=sc_t[:], axis=mybir.AxisListType.X)
                so_t = scp.tile([128, 1], F32, tag="sot")
                nc.vector.tensor_tensor(
                    out=so_t[:], in0=s_t[:], in1=bi_g[:, c : c + 1],
                    op=mybir.AluOpType.add,
                )
                nc.sync.dma_start(out=score[r0 : r0 + 128, :], in_=so_t[:])

    nc.compile()
    return nc


_NC_CACHE = {}


def _get_nc(key):
    if key not in _NC_CACHE:
        _NC_CACHE[key] = build_program(*key)
    return _NC_CACHE[key]


def make_in_maps(inputs, b_core=B_CORE, n_cores=N_CORES):
    """Shard inputs into per-core input maps (host side)."""
    w = _pack_weights(
        np.asarray(inputs["Wv"], np.float32),
        np.asarray(inputs["bv"], np.float32),
        np.asarray(inputs["Wh"], np.float32),
        np.asarray(inputs["bh"], np.float32),
        np.asarray(inputs["Wfc"], np.float32),
        np.asarray(inputs["bfc"], np.float32),
    )
    Q = np.ascontiguousarray(np.asarray(inputs["Q"], np.float32).astype(NP_CDT))
    P = np.ascontiguousarray(np.asarray(inputs["P"], np.float32))
    Qp = np.ascontiguousarray(np.asarray(inputs["Qp"], np.float32))
    bi = np.ascontiguousarray(
        np.asarray(inputs["b_item"], np.float32).reshape(-1, 1)
    )
    seq = np.asarray(inputs["seq_L"]).astype(np.int32)  # [B, L]
    user = np.asarray(inputs["user_id"]).astype(np.int32)
    item = np.asarray(inputs["item_id"]).astype(np.int32)

    in_maps = []
    for ci in range(n_cores):
        s = slice(ci * b_core, (ci + 1) * b_core)
        m = dict(w)
        m["Q"] = Q
        m["P"] = P
        m["Qp"] = Qp
        m["b_item"] = bi
        m["seqT32"] = np.ascontiguousarray(seq[s].T)  # [L, b_core]
        m["user32"] = np.ascontiguousarray(user[s])
        m["item32"] = np.ascontiguousarray(item[s])
        in_maps.append(m)
    return in_maps


def kernel(**inputs):
    nc = _get_nc((B_CORE, BT, NUM_ITEMS, NUM_USERS))
    in_maps = make_in_maps(inputs)
    res = run_bass_kernel_spmd(nc, in_maps, core_ids=list(range(N_CORES)))
    out = np.concatenate(
        [np.asarray(res.results[i]["score"]).reshape(-1) for i in range(N_CORES)]
    )
    return out.astype(np.float32)
